# Initial kernel scaffold
#
"""ConstituencyAwareAttention Trainium2 kernel.

Strategy: pure data parallelism -- B=8 batch elements across 8 NeuronCores,
one full attention problem per core (S=1024, H=1024, nh=16, hd=64). No
collectives.

Per-core pipeline (fp16 matmul inputs, fp32 accumulation):
  1. X^T is uploaded pre-transposed from the host (pure layout change, part
     of sharding) and cast to fp16 on device.
  2. V projection upfront (natural [S, h_out] layout, assembled as V_aug
     with a ones column per head so A@V also produces the softmax
     denominator). Q^T/K^T projections are pipelined per head-pair INTO the
     attention loop: chunk i of Q/K feeds head pair i. This keeps the PE
     saturated with dense 512-wide streams throughout, which keeps the HAM
     clock gate at 2.4 GHz (a PE starved below ~its activity window
     re-throttles to 1.2 GHz).
  3. scores^T = K_h @ Q_h^T / 8: two K=64 matmuls run concurrently in the
     PE via tile_position row packing (rows 0:63 even head, 64:127 odd
     head) -> full PE utilization despite hd=64.
  4. Constituency penalty folded into the exp bias: probs' = exp(s - 0.5),
     then the same-block [64,64] squares are multiplied by e^0.5. The mask
     is never materialized.
  5. AV flipped: ctx^T[hd+1, q] = V_aug.T @ probs^T accumulated over
     k-tiles; probs (the big operand) streams through the fast MM port
     instead of the slow LDWEIGHTS port. The small [65, 512] ctx^T tiles
     are PE-transposed back; row 64 is the denominator; normalize with
     per-partition reciprocal + scalar mul.
"""

import math
import sys

if "/opt/trn_rl_repo" not in sys.path:
    sys.path.insert(0, "/opt/trn_rl_repo")

import numpy as np

import concourse.bacc as bacc
import concourse.tile as tile
from concourse import mybir
from concourse.bass_utils import run_bass_kernel_spmd
from concourse.masks import make_identity

F16 = mybir.dt.float16
F32 = mybir.dt.float32

B, S, H = 8, 1024, 1024
NH, HD = 16, 64
P = 128
SO = S // P   # 8 S-chunks
KO = H // P   # 8 contraction chunks
PEN = 0.5
FIX = float(math.exp(PEN))
SCALE = 1.0 / math.sqrt(HD)

_programs = {}


def _build_program(with_bv: bool):
    nc = bacc.Bacc("TRN2", target_bir_lowering=False, debug=False)

    xt = nc.dram_tensor("xt", [H, S], F32, kind="ExternalInput").ap()
    wq = nc.dram_tensor("wq", [H, H], F32, kind="ExternalInput").ap()
    wk = nc.dram_tensor("wk", [H, H], F32, kind="ExternalInput").ap()
    wv = nc.dram_tensor("wv", [H, H], F32, kind="ExternalInput").ap()
    bq = nc.dram_tensor("bq", [H], F32, kind="ExternalInput").ap()
    bk = nc.dram_tensor("bk", [H], F32, kind="ExternalInput").ap()
    bv = nc.dram_tensor("bv", [H], F32, kind="ExternalInput").ap()
    out = nc.dram_tensor("out", [S, H], F32, kind="ExternalOutput").ap()

    Exp = mybir.ActivationFunctionType.Exp
    Identity = mybir.ActivationFunctionType.Identity

    with tile.TileContext(nc) as tc:
        with tc.tile_pool(name="persist", bufs=1) as persist:
            XT = persist.tile([P, KO, S], F16, name="XT")
            QT = persist.tile([P, KO, S], F16, name="QT")
            KT = persist.tile([P, KO, S], F16, name="KT")
            VA = persist.tile([P, SO, NH * (HD + 1)], F16, name="VA")
            ident = persist.tile([P, P], F16, name="ident")
            nbias = persist.tile([P, 1], F32, name="nbias")
            bq_s = persist.tile([P, KO], F32, name="bq_s")
            bk_s = persist.tile([P, KO], F32, name="bk_s")

            make_identity(nc, ident[:])
            nc.vector.memset(nbias[:], -PEN)
            # ones columns of V_aug prefilled; V block copies leave them alone
            nc.vector.memset(VA[:], 1.0)

            nc.sync.dma_start(out=bq_s[:], in_=bq.rearrange("(o p) -> p o", p=P))
            nc.vector.tensor_scalar_mul(bq_s[:], bq_s[:], SCALE)
            nc.sync.dma_start(out=bk_s[:], in_=bk.rearrange("(o p) -> p o", p=P))

            wctx = tc.tile_pool(name="wpool", bufs=1)
            wpool = wctx.__enter__()
            wqh = wpool.tile([P, KO, H], F16, name="wqh")
            wkh = wpool.tile([P, KO, H], F16, name="wkh")
            wvh = wpool.tile([P, KO, H], F16, name="wvh")

            with tc.tile_pool(name="stage", bufs=2) as stage:
                # X^T and Wv first: they gate the V projection, which runs
                # before the attention pipeline.
                xt32 = stage.tile([P, KO, S], F32, name="xt32", tag="stage32")
                nc.sync.dma_start(
                    out=xt32[:], in_=xt.rearrange("(ho hp) s -> hp ho s", hp=P)
                )
                nc.vector.tensor_copy(XT[:], xt32[:])
                for wsrc, wdst, dma_eng, eng in (
                    (wv, wvh, nc.scalar, "s"),
                    (wq, wqh, nc.gpsimd, "s"),
                    (wk, wkh, nc.sync, "v"),
                ):
                    w32 = stage.tile([P, KO, H], F32, name="w32", tag="stage32")
                    dma_eng.dma_start(
                        out=w32[:], in_=wsrc.rearrange("(kp_o kp) n -> kp kp_o n", kp=P)
                    )
                    if eng == "v":
                        nc.vector.tensor_copy(wdst[:], w32[:])
                    else:
                        nc.scalar.copy(wdst[:], w32[:])

            with (
                tc.tile_pool(name="attn", bufs=1) as attn,
                tc.tile_pool(name="proj_ps", bufs=1, space="PSUM") as proj_ps,
                tc.tile_pool(name="score_ps", bufs=2, space="PSUM") as score_ps,
                tc.tile_pool(name="ctx_ps", bufs=2, space="PSUM") as ctx_ps,
                tc.tile_pool(name="tp_ps", bufs=1, space="PSUM") as tp_ps,
            ):
                # ---- V projection: natural [S, h_out] -> V_aug blocks ----
                for so in range(SO):
                    for ncol in range(2):
                        ps = proj_ps.tile([P, 512], F32, name="ps", tag="proj")
                        for kh in range(KO):
                            nc.tensor.matmul(
                                ps[:],
                                XT[:, kh, so * P : (so + 1) * P],
                                wvh[:, kh, ncol * 512 : (ncol + 1) * 512],
                                start=(kh == 0),
                                stop=(kh == KO - 1),
                            )
                        va_v = VA[:, so, :].rearrange("p (h c) -> p h c", c=HD + 1)
                        nc.vector.tensor_copy(
                            va_v[:, ncol * 8 : (ncol + 1) * 8, 0:HD],
                            ps[:].rearrange("p (h c) -> p h c", c=HD),
                        )

                if with_bv:
                    # out += bv exactly (softmax rows sum to 1), via a
                    # PE-broadcast of bv across partitions added to V_aug.
                    ones1 = persist.tile([1, P], F16, name="ones1")
                    nc.vector.memset(ones1[:], 1.0)
                    bv1 = persist.tile([1, H], F16, name="bv1")
                    bv1_32 = persist.tile([1, H], F32, name="bv1_32")
                    nc.sync.dma_start(out=bv1_32[:], in_=bv[None, :])
                    nc.vector.tensor_copy(bv1[:], bv1_32[:])
                    bvb = persist.tile([P, NH * (HD + 1)], F16, name="bvb")
                    nc.vector.memset(bvb[:], 0.0)
                    bvb_v = bvb.rearrange("p (h c) -> p h c", c=HD + 1)
                    for ncol in range(2):
                        psb = proj_ps.tile([P, 512], F32, name="psb", tag="proj")
                        nc.tensor.matmul(
                            psb[:], ones1[:], bv1[:, ncol * 512 : (ncol + 1) * 512],
                            start=True, stop=True,
                        )
                        nc.vector.tensor_copy(
                            bvb_v[:, ncol * 8 : (ncol + 1) * 8, 0:HD],
                            psb[:].rearrange("p (h c) -> p h c", c=HD),
                        )
                    for so in range(SO):
                        nc.vector.tensor_add(VA[:, so, :], VA[:, so, :], bvb[:])

                # ---- Q/K projection for one h_out chunk (= head pair mo) ----
                def qk_proj_chunk(mo):
                    for wsb, dst in ((wqh, QT), (wkh, KT)):
                        for sc in range(2):
                            ps = proj_ps.tile([P, 512], F32, name="ps", tag="proj")
                            for kh in range(KO):
                                nc.tensor.matmul(
                                    ps[:],
                                    wsb[:, kh, mo * P : (mo + 1) * P],
                                    XT[:, kh, sc * 512 : (sc + 1) * 512],
                                    start=(kh == 0),
                                    stop=(kh == KO - 1),
                                )
                            if dst is QT:
                                nc.vector.tensor_scalar(
                                    QT[:, mo, sc * 512 : (sc + 1) * 512], ps[:],
                                    SCALE, bq_s[:, mo : mo + 1],
                                    mybir.AluOpType.mult, mybir.AluOpType.add,
                                )
                            else:
                                nc.vector.tensor_scalar_add(
                                    KT[:, mo, sc * 512 : (sc + 1) * 512], ps[:],
                                    bk_s[:, mo : mo + 1],
                                )

                out_sb = attn.tile([P, SO, H], F32, name="out_sb")
                out_r = out.rearrange("(o p) n -> p o n", p=P)
                qk_proj_chunk(0)

                for i in range(NH // 2):
                    prT = [
                        attn.tile([P, KO, S], F16, name="prT", tag="probsT", bufs=3)
                        for _ in range(2)
                    ]
                    # scores + exp for pair i
                    for kt in range(KO):
                        pst = [
                            score_ps.tile([P, S], F32, name="pst", tag="score")
                            for _ in range(2)
                        ]
                        for qc in range(2):
                            for half in range(2):
                                lo = half * 64
                                nc.tensor.matmul(
                                    pst[half][:, qc * 512 : (qc + 1) * 512],
                                    KT[lo : lo + 64, i, kt * P : (kt + 1) * P],
                                    QT[lo : lo + 64, i, qc * 512 : (qc + 1) * 512],
                                    start=True,
                                    stop=True,
                                    tile_position=(lo, 0),
                                )
                        for half in range(2):
                            nc.scalar.activation(
                                prT[half][:, kt, :], pst[half][:], Exp, bias=nbias[:]
                            )
                            # same-constituent squares: undo the -0.5 penalty
                            nc.vector.tensor_scalar_mul(
                                prT[half][0:64, kt, kt * P : kt * P + 64],
                                prT[half][0:64, kt, kt * P : kt * P + 64],
                                FIX,
                            )
                            nc.vector.tensor_scalar_mul(
                                prT[half][64:128, kt, kt * P + 64 : (kt + 1) * P],
                                prT[half][64:128, kt, kt * P + 64 : (kt + 1) * P],
                                FIX,
                            )

                    # next pair's Q/K projection: dense PE filler that also
                    # hides the exp latency of this pair
                    if i + 1 < NH // 2:
                        qk_proj_chunk(i + 1)

                    # AV for pair i (flipped); the PE transposes of each
                    # (h, qc) group are emitted interleaved with the NEXT
                    # group's AV matmuls so transpose-mode activity (which
                    # the HAM clock gate does not count as busy) never
                    # clusters into a full throttle window.
                    def emit_transposes(h, qc, ctxt_sb):
                        for c4 in range(4):
                            so = qc * 4 + c4
                            tp = tp_ps.tile([P, HD + 1], F16, name="tp", tag="tp")
                            nc.tensor.transpose(
                                tp[:],
                                ctxt_sb[:, c4 * P : (c4 + 1) * P],
                                ident[0 : HD + 1, 0 : HD + 1],
                            )
                            inv = attn.tile(
                                [P, 1], F32, name="inv", tag="inv", bufs=8
                            )
                            nc.vector.reciprocal(inv[:], tp[:, HD : HD + 1])
                            nc.vector.tensor_scalar_mul(
                                out_sb[:, so, h * HD : (h + 1) * HD],
                                tp[:, 0:HD],
                                inv[:],
                            )

                    pending = None
                    for half in range(2):
                        h = 2 * i + half
                        for qc in range(2):
                            ctxt = ctx_ps.tile(
                                [HD + 1, 512], F32, name="ctxt", tag="ctxt"
                            )
                            for kt in range(KO):
                                nc.tensor.matmul(
                                    ctxt[:],
                                    VA[:, kt, h * (HD + 1) : (h + 1) * (HD + 1)],
                                    prT[half][:, kt, qc * 512 : (qc + 1) * 512],
                                    start=(kt == 0),
                                    stop=(kt == KO - 1),
                                )
                            ctxt_sb = attn.tile(
                                [HD + 1, 512], F16, name="ctxt_sb", tag="ctxt_sb",
                                bufs=4,
                            )
                            nc.vector.tensor_copy(ctxt_sb[:], ctxt[:])
                            if pending is not None:
                                emit_transposes(*pending)
                            pending = (h, qc, ctxt_sb)
                    emit_transposes(*pending)
                    # pair i's output columns are complete -> ship them while
                    # the next pair computes
                    nc.sync.dma_start(
                        out=out_r[:, :, i * P : (i + 1) * P],
                        in_=out_sb[:, :, i * P : (i + 1) * P],
                    )


            wctx.__exit__(None, None, None)

    nc.compile()
    return nc


def _get_program(with_bv: bool):
    key = with_bv
    if key not in _programs:
        _programs[key] = _build_program(with_bv)
    return _programs[key]


def _in_maps(hidden_states, Wq, bq, Wk, bk, Wv, bv):
    wq = np.ascontiguousarray(Wq, np.float32)
    wk = np.ascontiguousarray(Wk, np.float32)
    wv = np.ascontiguousarray(Wv, np.float32)
    bq = np.ascontiguousarray(bq, np.float32)
    bk = np.ascontiguousarray(bk, np.float32)
    bv = np.ascontiguousarray(bv, np.float32)
    return [
        {
            "xt": np.ascontiguousarray(hidden_states[b].T, np.float32),
            "wq": wq, "wk": wk, "wv": wv, "bq": bq, "bk": bk, "bv": bv,
        }
        for b in range(B)
    ]


def kernel(hidden_states, Wq, bq, Wk, bk, Wv, bv):
    hidden_states = np.ascontiguousarray(hidden_states, dtype=np.float32)
    with_bv = bool(np.any(np.asarray(bv) != 0))
    nc = _get_program(with_bv)
    in_maps = _in_maps(hidden_states, Wq, bq, Wk, bk, Wv, bv)
    last_err = None
    for _attempt in range(3):
        try:
            res = run_bass_kernel_spmd(nc, in_maps, list(range(B)))
            return np.stack([res.results[b]["out"] for b in range(B)], axis=0)
        except Exception as e:  # transient NRT device errors recover on retry
            last_err = e
            import time
            time.sleep(3)
    raise last_err



# revision 1
# speedup vs baseline: 1.2377x; 1.2377x over previous
"""ConstituencyAwareAttention Trainium2 kernel.

Strategy: pure data parallelism -- B=8 batch elements across 8 NeuronCores,
one full attention problem per core (S=1024, H=1024, nh=16, hd=64). No
collectives.

Per-core pipeline (fp16 matmul inputs, fp32 accumulation):
  1. X^T is uploaded pre-transposed from the host (pure layout change, part
     of sharding) and cast to fp16 on device.
  2. V projection upfront (natural [S, h_out] layout, assembled as V_aug
     with a ones column per head so A@V also produces the softmax
     denominator). Q^T/K^T projections are pipelined per head-pair INTO the
     attention loop: chunk i of Q/K feeds head pair i. This keeps the PE
     saturated with dense 512-wide streams throughout, which keeps the HAM
     clock gate at 2.4 GHz (a PE starved below ~its activity window
     re-throttles to 1.2 GHz).
  3. scores^T = K_h @ Q_h^T / 8: two K=64 matmuls run concurrently in the
     PE via tile_position row packing (rows 0:63 even head, 64:127 odd
     head) -> full PE utilization despite hd=64.
  4. Constituency penalty folded into the exp bias: probs' = exp(s - 0.5),
     then the same-block [64,64] squares are multiplied by e^0.5. The mask
     is never materialized.
  5. AV flipped: ctx^T[hd+1, q] = V_aug.T @ probs^T accumulated over
     k-tiles; probs (the big operand) streams through the fast MM port
     instead of the slow LDWEIGHTS port. The small [65, 512] ctx^T tiles
     are PE-transposed back; row 64 is the denominator; normalize with
     per-partition reciprocal + scalar mul.
"""

import math
import sys

if "/opt/trn_rl_repo" not in sys.path:
    sys.path.insert(0, "/opt/trn_rl_repo")

import numpy as np

import concourse.bacc as bacc
import concourse.tile as tile
from concourse import mybir
from concourse.bass_utils import run_bass_kernel_spmd
from concourse.masks import make_identity

F16 = mybir.dt.float16
F32 = mybir.dt.float32

B, S, H = 8, 1024, 1024
NH, HD = 16, 64
P = 128
SO = S // P   # 8 S-chunks
KO = H // P   # 8 contraction chunks
PEN = 0.5
FIX = float(math.exp(PEN))
SCALE = 1.0 / math.sqrt(HD)

_programs = {}


def _build_program(with_bv: bool):
    nc = bacc.Bacc("TRN2", target_bir_lowering=False, debug=False)

    xt = nc.dram_tensor("xt", [H, S], F32, kind="ExternalInput").ap()
    wq = nc.dram_tensor("wq", [H, H], F32, kind="ExternalInput").ap()
    wk = nc.dram_tensor("wk", [H, H], F32, kind="ExternalInput").ap()
    wv = nc.dram_tensor("wv", [H, H], F32, kind="ExternalInput").ap()
    bq = nc.dram_tensor("bq", [H], F32, kind="ExternalInput").ap()
    bk = nc.dram_tensor("bk", [H], F32, kind="ExternalInput").ap()
    bv = nc.dram_tensor("bv", [H], F32, kind="ExternalInput").ap()
    out = nc.dram_tensor("out", [S, H], F32, kind="ExternalOutput").ap()

    Exp = mybir.ActivationFunctionType.Exp
    Identity = mybir.ActivationFunctionType.Identity

    with tile.TileContext(nc) as tc:
        with tc.tile_pool(name="persist", bufs=1) as persist:
            XT = persist.tile([P, KO, S], F16, name="XT")
            QT = persist.tile([P, KO, S], F16, name="QT")
            KT = persist.tile([P, KO, S], F16, name="KT")
            VA = persist.tile([P, SO, NH * (HD + 1)], F16, name="VA")
            ident = persist.tile([P, P], F16, name="ident")
            nbias = persist.tile([P, 1], F32, name="nbias")
            bq_s = persist.tile([P, KO], F32, name="bq_s")
            bk_s = persist.tile([P, KO], F32, name="bk_s")

            make_identity(nc, ident[:])
            nc.vector.memset(nbias[:], -PEN)
            # ones columns of V_aug prefilled; V block copies leave them alone
            nc.vector.memset(VA[:], 1.0)

            nc.sync.dma_start(out=bq_s[:], in_=bq.rearrange("(o p) -> p o", p=P))
            nc.vector.tensor_scalar_mul(bq_s[:], bq_s[:], SCALE)
            nc.sync.dma_start(out=bk_s[:], in_=bk.rearrange("(o p) -> p o", p=P))

            wctx = tc.tile_pool(name="wpool", bufs=1)
            wpool = wctx.__enter__()
            wqh = wpool.tile([P, KO, H], F16, name="wqh")
            wkh = wpool.tile([P, KO, H], F16, name="wkh")
            wvh = wpool.tile([P, KO, H], F16, name="wvh")

            with tc.tile_pool(name="stage", bufs=2) as stage:
                # X^T and Wv first: they gate the V projection, which runs
                # before the attention pipeline.
                xt32 = stage.tile([P, KO, S], F32, name="xt32", tag="stage32")
                nc.sync.dma_start(
                    out=xt32[:], in_=xt.rearrange("(ho hp) s -> hp ho s", hp=P)
                )
                nc.vector.tensor_copy(XT[:], xt32[:])
                for wsrc, wdst, dma_eng, eng in (
                    (wv, wvh, nc.scalar, "s"),
                    (wq, wqh, nc.gpsimd, "s"),
                    (wk, wkh, nc.sync, "v"),
                ):
                    w32 = stage.tile([P, KO, H], F32, name="w32", tag="stage32")
                    dma_eng.dma_start(
                        out=w32[:], in_=wsrc.rearrange("(kp_o kp) n -> kp kp_o n", kp=P)
                    )
                    if eng == "v":
                        nc.vector.tensor_copy(wdst[:], w32[:])
                    else:
                        nc.scalar.copy(wdst[:], w32[:])

            with (
                tc.tile_pool(name="attn", bufs=1) as attn,
                tc.tile_pool(name="proj_ps", bufs=1, space="PSUM") as proj_ps,
                tc.tile_pool(name="score_ps", bufs=2, space="PSUM") as score_ps,
                tc.tile_pool(name="ctx_ps", bufs=2, space="PSUM") as ctx_ps,
                tc.tile_pool(name="tp_ps", bufs=1, space="PSUM") as tp_ps,
            ):
                # ---- V projection: natural [S, h_out] -> V_aug blocks ----
                for so in range(SO):
                    for ncol in range(2):
                        ps = proj_ps.tile([P, 512], F32, name="ps", tag="proj")
                        for kh in range(KO):
                            nc.tensor.matmul(
                                ps[:],
                                XT[:, kh, so * P : (so + 1) * P],
                                wvh[:, kh, ncol * 512 : (ncol + 1) * 512],
                                start=(kh == 0),
                                stop=(kh == KO - 1),
                            )
                        va_v = VA[:, so, :].rearrange("p (h c) -> p h c", c=HD + 1)
                        nc.vector.tensor_copy(
                            va_v[:, ncol * 8 : (ncol + 1) * 8, 0:HD],
                            ps[:].rearrange("p (h c) -> p h c", c=HD),
                        )

                if with_bv:
                    # out += bv exactly (softmax rows sum to 1), via a
                    # PE-broadcast of bv across partitions added to V_aug.
                    ones1 = persist.tile([1, P], F16, name="ones1")
                    nc.vector.memset(ones1[:], 1.0)
                    bv1 = persist.tile([1, H], F16, name="bv1")
                    bv1_32 = persist.tile([1, H], F32, name="bv1_32")
                    nc.sync.dma_start(out=bv1_32[:], in_=bv[None, :])
                    nc.vector.tensor_copy(bv1[:], bv1_32[:])
                    bvb = persist.tile([P, NH * (HD + 1)], F16, name="bvb")
                    nc.vector.memset(bvb[:], 0.0)
                    bvb_v = bvb.rearrange("p (h c) -> p h c", c=HD + 1)
                    for ncol in range(2):
                        psb = proj_ps.tile([P, 512], F32, name="psb", tag="proj")
                        nc.tensor.matmul(
                            psb[:], ones1[:], bv1[:, ncol * 512 : (ncol + 1) * 512],
                            start=True, stop=True,
                        )
                        nc.vector.tensor_copy(
                            bvb_v[:, ncol * 8 : (ncol + 1) * 8, 0:HD],
                            psb[:].rearrange("p (h c) -> p h c", c=HD),
                        )
                    for so in range(SO):
                        nc.vector.tensor_add(VA[:, so, :], VA[:, so, :], bvb[:])

                # ---- Q/K projection for one h_out chunk (= head pair mo) ----
                def qk_proj_chunk(mo):
                    for wsb, dst in ((wqh, QT), (wkh, KT)):
                        for sc in range(2):
                            ps = proj_ps.tile([P, 512], F32, name="ps", tag="proj")
                            for kh in range(KO):
                                nc.tensor.matmul(
                                    ps[:],
                                    wsb[:, kh, mo * P : (mo + 1) * P],
                                    XT[:, kh, sc * 512 : (sc + 1) * 512],
                                    start=(kh == 0),
                                    stop=(kh == KO - 1),
                                )
                            if dst is QT:
                                nc.vector.tensor_scalar(
                                    QT[:, mo, sc * 512 : (sc + 1) * 512], ps[:],
                                    SCALE, bq_s[:, mo : mo + 1],
                                    mybir.AluOpType.mult, mybir.AluOpType.add,
                                )
                            else:
                                nc.vector.tensor_scalar_add(
                                    KT[:, mo, sc * 512 : (sc + 1) * 512], ps[:],
                                    bk_s[:, mo : mo + 1],
                                )

                out_sb = attn.tile([P, SO, H], F32, name="out_sb")
                out_r = out.rearrange("(o p) n -> p o n", p=P)
                qk_proj_chunk(0)

                for i in range(NH // 2):
                    prT = [
                        attn.tile([P, KO, S], F16, name="prT", tag="probsT", bufs=3)
                        for _ in range(2)
                    ]
                    # scores + exp for pair i
                    for kt in range(KO):
                        pst = [
                            score_ps.tile([P, S], F32, name="pst", tag="score")
                            for _ in range(2)
                        ]
                        for qc in range(2):
                            for half in range(2):
                                lo = half * 64
                                nc.tensor.matmul(
                                    pst[half][:, qc * 512 : (qc + 1) * 512],
                                    KT[lo : lo + 64, i, kt * P : (kt + 1) * P],
                                    QT[lo : lo + 64, i, qc * 512 : (qc + 1) * 512],
                                    start=True,
                                    stop=True,
                                    tile_position=(lo, 0),
                                )
                        for half in range(2):
                            nc.scalar.activation(
                                prT[half][:, kt, :], pst[half][:], Exp, bias=nbias[:]
                            )
                            # same-constituent squares: undo the -0.5 penalty
                            nc.vector.tensor_scalar_mul(
                                prT[half][0:64, kt, kt * P : kt * P + 64],
                                prT[half][0:64, kt, kt * P : kt * P + 64],
                                FIX,
                            )
                            nc.vector.tensor_scalar_mul(
                                prT[half][64:128, kt, kt * P + 64 : (kt + 1) * P],
                                prT[half][64:128, kt, kt * P + 64 : (kt + 1) * P],
                                FIX,
                            )

                    # next pair's Q/K projection: dense PE filler that also
                    # hides the exp latency of this pair
                    if i + 1 < NH // 2:
                        qk_proj_chunk(i + 1)

                    # AV for pair i (flipped); the PE transposes of each
                    # (h, qc) group are emitted interleaved with the NEXT
                    # group's AV matmuls so transpose-mode activity (which
                    # the HAM clock gate does not count as busy) never
                    # clusters into a full throttle window.
                    def emit_transposes(h, qc, ctxt_sb):
                        for c4 in range(4):
                            so = qc * 4 + c4
                            tp = tp_ps.tile([P, HD + 1], F16, name="tp", tag="tp")
                            nc.tensor.transpose(
                                tp[:],
                                ctxt_sb[:, c4 * P : (c4 + 1) * P],
                                ident[0 : HD + 1, 0 : HD + 1],
                            )
                            inv = attn.tile(
                                [P, 1], F32, name="inv", tag="inv", bufs=8
                            )
                            nc.vector.reciprocal(inv[:], tp[:, HD : HD + 1])
                            nc.vector.tensor_scalar_mul(
                                out_sb[:, so, h * HD : (h + 1) * HD],
                                tp[:, 0:HD],
                                inv[:],
                            )

                    pending = None
                    for half in range(2):
                        h = 2 * i + half
                        for qc in range(2):
                            ctxt = ctx_ps.tile(
                                [HD + 1, 512], F32, name="ctxt", tag="ctxt"
                            )
                            for kt in range(KO):
                                nc.tensor.matmul(
                                    ctxt[:],
                                    VA[:, kt, h * (HD + 1) : (h + 1) * (HD + 1)],
                                    prT[half][:, kt, qc * 512 : (qc + 1) * 512],
                                    start=(kt == 0),
                                    stop=(kt == KO - 1),
                                )
                            ctxt_sb = attn.tile(
                                [HD + 1, 512], F16, name="ctxt_sb", tag="ctxt_sb",
                                bufs=4,
                            )
                            nc.vector.tensor_copy(ctxt_sb[:], ctxt[:])
                            if pending is not None:
                                emit_transposes(*pending)
                            pending = (h, qc, ctxt_sb)
                    emit_transposes(*pending)
                    # pair i's output columns are complete -> ship them while
                    # the next pair computes
                    nc.sync.dma_start(
                        out=out_r[:, :, i * P : (i + 1) * P],
                        in_=out_sb[:, :, i * P : (i + 1) * P],
                    )


            wctx.__exit__(None, None, None)

    nc.compile()
    return nc


def _get_program(with_bv: bool):
    key = with_bv
    if key not in _programs:
        _programs[key] = _build_program(with_bv)
    return _programs[key]


def _in_maps(hidden_states, Wq, bq, Wk, bk, Wv, bv):
    wq = np.ascontiguousarray(Wq, np.float32)
    wk = np.ascontiguousarray(Wk, np.float32)
    wv = np.ascontiguousarray(Wv, np.float32)
    bq = np.ascontiguousarray(bq, np.float32)
    bk = np.ascontiguousarray(bk, np.float32)
    bv = np.ascontiguousarray(bv, np.float32)
    return [
        {
            "xt": np.ascontiguousarray(hidden_states[b].T, np.float32),
            "wq": wq, "wk": wk, "wv": wv, "bq": bq, "bk": bk, "bv": bv,
        }
        for b in range(B)
    ]


def kernel(hidden_states, Wq, bq, Wk, bk, Wv, bv):
    hidden_states = np.ascontiguousarray(hidden_states, dtype=np.float32)
    with_bv = bool(np.any(np.asarray(bv) != 0))
    nc = _get_program(with_bv)
    in_maps = _in_maps(hidden_states, Wq, bq, Wk, bk, Wv, bv)
    last_err = None
    for _attempt in range(3):
        try:
            res = run_bass_kernel_spmd(nc, in_maps, list(range(B)))
            return np.stack([res.results[b]["out"] for b in range(B)], axis=0)
        except Exception as e:  # transient NRT device errors recover on retry
            last_err = e
            import time
            time.sleep(3)
    raise last_err

